# revision 24
# baseline (speedup 1.0000x reference)
"""GroupedQueryAttention (B=2, S=2048, DIM=1024, 16 heads, 4 KV groups) on 8 trn2 cores.

Sharding: core c -> (batch b = c // 4, kv-group g = c % 4).
Each core: LayerNorm(x[b]) -> q/k/v projections for group g -> attention for the
group's 4 heads -> partial out-projection (w_o rows for group g), producing
y_c = partial_out^T [DIM, S].  Host sums the 4 group partials per batch,
transposes, and adds b_o.

v8 (final): 450us (v1) -> 329 -> 278 -> 271 -> 262 -> 259us HW exec.
- fp16 matmul operands everywhere (full PE rate; f32r weight loads serialize,
  fp16 gets background fast-weight-load). PSUM accumulation stays f32;
  LayerNorm statistics stay f32. Final rel err ~6e-4.
- Attention is one software-pipelined stream: scores(t) and exp(t) are emitted
  before the lagged pv(t-1), across iteration boundaries, so the Scalar
  engine's 128 exp tiles ([128,1024] each, the true bottleneck at ~147us) run
  back-to-back with ~5us total gap.
- Scores contraction is zero-padded to K=128 (q/k carry a zeroed complement
  half; odd heads live in partitions 64:128) so every matmul runs in the full
  128x128 array mode - no tiling-mode thrash - and all weights are 128-col.
- PV keeps the ones-column denominator trick (padded to 128 weight cols);
  PSUM: 2x2-bank score buffers + 2x(1+1)-bank PV accumulators = 8 banks.
- Softmax normalization off the critical path: denominator row -> SBUF copy ->
  fast approximate reciprocal (custom DVE op; NaN if fed PSUM directly) ->
  gpsimd partition-broadcast -> multiply.
- Phase 1 balances ACT (squares, sqrt, odd-half evictions, half the xnT
  copies) against DVE (mean-reduce, xn, stats smalls, even-half evictions);
  x tiles DMA in pairs; weights load as single batched DMAs; first x tiles
  are prefetched before the weight DMAs.
- Out-projection tail is chunked into [128,1024] PSUM tiles (bufs=3) with
  alternating ACT/DVE evictions.
"""

import numpy as np

import concourse.bass as bass
import concourse.mybir as mybir
from concourse import bacc
from concourse.bass_utils import run_bass_kernel_spmd
from concourse.tile import TileContext
from concourse.masks import make_identity

B, S, DIM = 2, 2048, 1024
HEADS, DH, G = 16, 64, 4
HPG = HEADS // G              # 4 heads per group
EG = HPG * DH                 # 256 q columns per group
SCALE = DH ** -0.5
P = 128
NT_S = S // P                 # 16
NT_D = DIM // P               # 8
F32 = mybir.dt.float32
FP16 = mybir.dt.float16
AF = mybir.ActivationFunctionType
OP = mybir.AluOpType
AX = mybir.AxisListType


def build_nc():
    nc = bacc.Bacc("TRN2", target_bir_lowering=False)
    MMDT = FP16
    x = nc.dram_tensor("x", [S, DIM], F32, kind="ExternalInput")
    wq = nc.dram_tensor("wq", [DIM, EG], MMDT, kind="ExternalInput")
    wkv = nc.dram_tensor("wkv", [DIM, 2 * DH], MMDT, kind="ExternalInput")
    wo = nc.dram_tensor("wo", [EG, DIM], MMDT, kind="ExternalInput")
    qb = nc.dram_tensor("qb", [2, P], F32, kind="ExternalInput")   # beta @ w_q slice
    kvb = nc.dram_tensor("kvb", [1, P], F32, kind="ExternalInput")  # beta @ [w_k|w_v] slice
    y = nc.dram_tensor("y", [DIM, S], F32, kind="ExternalOutput")

    with TileContext(nc) as tc:
        with tc.tile_pool(name="persist", bufs=1) as pp:
            with tc.tile_pool(name="xin", bufs=6) as xip:
                # prefetch first x tiles before the weight DMAs
                xts = {}
                for i2 in range(2):
                    xt2 = xip.tile([P, 2, DIM], F32, tag="x")
                    nc.sync.dma_start(
                        out=xt2[:],
                        in_=x[i2 * 2 * P:(i2 + 1) * 2 * P, :].rearrange(
                            "(c p) d -> p c d", p=P))
                    xts[i2] = xt2

                ident = pp.tile([P, P], F32)
                make_identity(nc, ident[:])
                identr = pp.tile([P, P], MMDT)
                nc.vector.tensor_copy(out=identr[:], in_=ident[:])

                wq_sb = pp.tile([P, NT_D, EG], MMDT)
                wkv_sb = pp.tile([P, NT_D, 2 * DH], MMDT)
                wo_sb = pp.tile([P, 2, DIM], MMDT)
                qb_sb = pp.tile([P, 2], F32)
                kvb_sb = pp.tile([P, 1], F32)
                nc.sync.dma_start(
                    out=wq_sb[:], in_=wq.rearrange("(c p) e -> p c e", p=P))
                nc.sync.dma_start(
                    out=wkv_sb[:], in_=wkv.rearrange("(c p) e -> p c e", p=P))
                nc.sync.dma_start(
                    out=wo_sb[:], in_=wo.rearrange("(c p) e -> p c e", p=P))
                nc.sync.dma_start(out=qb_sb[:], in_=qb.rearrange("c p -> p c"))
                nc.sync.dma_start(out=kvb_sb[:], in_=kvb.rearrange("c p -> p c"))

                eps_sb = pp.tile([P, 1], F32)
                nc.vector.memset(eps_sb[:], 1e-5)

                # persistent activation layout; qTz slot h keeps head data in
                # its native partition half, zeros in the other (K-padding).
                qTz = pp.tile([P, HPG, S], MMDT)
                kz = pp.tile([P, S], MMDT)     # k^T rows 0:64, zeros 64:128
                kdupz = pp.tile([P, S], MMDT)  # zeros 0:64, k^T copy 64:128
                vsb = pp.tile([P, S], MMDT)    # v^T rows 64:128; 0:64 scratch
                vones = pp.tile([P, NT_S, P], MMDT)  # V nat | ones | zero pad
                outT = pp.tile([P, 2, S], MMDT)

                for h in range(HPG):
                    z0, z1 = (DH, P) if h % 2 == 0 else (0, DH)
                    nc.gpsimd.memset(qTz[z0:z1, h, :], 0.0)
                nc.gpsimd.memset(kz[DH:P, :], 0.0)
                nc.gpsimd.memset(kdupz[0:DH, :], 0.0)
                nc.vector.memset(vones[:], 0.0)
                ones_col = pp.tile([P, 1], MMDT)
                nc.vector.memset(ones_col[:], 1.0)
                nc.vector.tensor_copy(out=vones[:, :, DH],
                                      in_=ones_col[:].broadcast_to([P, NT_S]))

                # ---- attention pipeline machinery (used in both segments) ----
                # iteration k -> (hp, c) = (k % 2, k // 2)
                pss, ess, pos = {}, {}, {}
                pvn = {}
                pending = []
                state = {"psp": None, "pop": None, "ap": None, "bp": None}

                def sq_of(k):
                    return slice((k // 2) * 512, (k // 2) * 512 + 512)

                def scores(k, m):
                    hp = k % 2
                    ps = state["psp"].tile([P, 1024], F32, tag="ps")
                    nc.tensor.matmul(
                        ps[:, 0:512],
                        lhsT=kz[:, m * P:(m + 1) * P],
                        rhs=qTz[:, 2 * hp, sq_of(k)], start=True, stop=True)
                    nc.tensor.matmul(
                        ps[:, 512:1024],
                        lhsT=kdupz[:, m * P:(m + 1) * P],
                        rhs=qTz[:, 2 * hp + 1, sq_of(k)], start=True, stop=True)
                    pss[(k, m)] = ps

                def expm(k, m):
                    es = state["ap"].tile([P, 1024], MMDT, tag="es")
                    nc.scalar.activation(es[:], pss.pop((k, m))[:], AF.Exp,
                                         scale=SCALE)
                    ess[(k, m)] = es

                def norm(k):
                    hp, sq = k % 2, sq_of(k)
                    po_e, po_o = pos.pop(k)
                    bp = state["bp"]
                    den_e = bp.tile([1, 512], F32, tag="dene")
                    nc.vector.tensor_copy(out=den_e[:], in_=po_e[DH:DH + 1, :])
                    rc_e = bp.tile([1, 512], F32, tag="rce")
                    nc.vector.reciprocal_approx_fast(rc_e[:], den_e[:])
                    db_e = bp.tile([DH, 512], F32, tag="dbe")
                    nc.gpsimd.partition_broadcast(db_e[:], rc_e[:])
                    nc.vector.tensor_mul(outT[0:DH, hp, sq],
                                         po_e[0:DH, :], db_e[:])
                    den_o = bp.tile([1, 512], F32, tag="deno")
                    nc.vector.tensor_copy(out=den_o[:], in_=po_o[DH:DH + 1, :])
                    rc_o = bp.tile([1, 512], F32, tag="rco")
                    nc.vector.reciprocal_approx_fast(rc_o[:], den_o[:])
                    db_o = bp.tile([DH, 512], F32, tag="dbo")
                    nc.gpsimd.partition_broadcast(db_o[:], rc_o[:])
                    st = bp.tile([DH, 512], MMDT, tag="st")
                    nc.vector.tensor_mul(st[:], po_o[0:DH, :], db_o[:])
                    nc.sync.dma_start(out=outT[DH:P, hp, sq], in_=st[:])

                def pv(k, m):
                    if m == 0:
                        po_e = state["pop"].tile([P, 512], F32, tag="poe")
                        po_o = state["pop"].tile([P, 512], F32, tag="poo")
                        pos[k] = (po_e, po_o)
                        pvn[k] = 0
                    po_e, po_o = pos[k]
                    es = ess.pop((k, m))
                    nc.tensor.matmul(
                        po_e[:], lhsT=vones[:, m, :], rhs=es[:, 0:512],
                        start=(m == 0), stop=(m == NT_S - 1))
                    nc.tensor.matmul(
                        po_o[:], lhsT=vones[:, m, :], rhs=es[:, 512:1024],
                        start=(m == 0), stop=(m == NT_S - 1))
                    pvn[k] += 1
                    if pvn[k] == NT_S:
                        norm(k)

                def emit_step(k, m):
                    scores(k, m)
                    expm(k, m)
                    if pending:
                        pv(*pending.pop())
                    pending.append((k, m))

                def flush_pv():
                    while pending:
                        pv(*pending.pop())

                # ---------- Phase 1 + interleaved attention (k = 0, 1) ----------
                with tc.tile_pool(name="xnTp", bufs=1) as xp:
                    xnT = xp.tile([P, NT_D, S], MMDT)
                    with tc.tile_pool(name="ln", bufs=4) as lnp, \
                         tc.tile_pool(name="scr", bufs=3) as scp, \
                         tc.tile_pool(name="lns", bufs=3) as lsp, \
                         tc.tile_pool(name="psT", bufs=2, space="PSUM") as ptp, \
                         tc.tile_pool(name="psP", bufs=4, space="PSUM") as ppp:
                        for quarter in range(4):
                            for i in range(quarter * 4, quarter * 4 + 4):
                                i2, sub = divmod(i, 2)
                                if sub == 0:
                                    if i2 in xts:
                                        xt2 = xts.pop(i2)
                                    else:
                                        xt2 = xip.tile([P, 2, DIM], F32, tag="x")
                                        nc.sync.dma_start(
                                            out=xt2[:],
                                            in_=x[i2 * 2 * P:(i2 + 1) * 2 * P, :]
                                            .rearrange("(c p) d -> p c d", p=P))
                                    last_xt2 = xt2
                                xt = last_xt2[:, sub, :]
                                sm = lsp.tile([P, 1], F32, tag="sm")
                                nc.vector.tensor_reduce(
                                    out=sm[:], in_=xt, axis=AX.X, op=OP.add)
                                scr = scp.tile([P, DIM], F32, tag="scr")
                                ssq = lsp.tile([P, 1], F32, tag="ssq")
                                nc.scalar.activation(scr[:], xt, AF.Square,
                                                     accum_out=ssq[:])
                                msq = lsp.tile([P, 1], F32, tag="msq")
                                nc.vector.tensor_scalar(
                                    out=msq[:], in0=ssq[:], scalar1=1.0 / DIM,
                                    scalar2=None, op0=OP.mult)
                                negmu = lsp.tile([P, 1], F32, tag="negmu")
                                nc.vector.tensor_scalar(
                                    out=negmu[:], in0=sm[:], scalar1=-1.0 / DIM,
                                    scalar2=None, op0=OP.mult)
                                mu2 = lsp.tile([P, 1], F32, tag="mu2")
                                nc.vector.tensor_mul(mu2[:], negmu[:], negmu[:])
                                var = lsp.tile([P, 1], F32, tag="var")
                                nc.vector.tensor_sub(var[:], msq[:], mu2[:])
                                std = lsp.tile([P, 1], F32, tag="std")
                                nc.scalar.activation(std[:], var[:], AF.Sqrt,
                                                     bias=eps_sb[:])
                                rstd = lsp.tile([P, 1], F32, tag="rstd")
                                nc.vector.reciprocal(rstd[:], std[:])
                                xn = lnp.tile([P, DIM], MMDT, tag="xn")
                                if i % 4 == 3:
                                    # out = Identity(x*rstd + (negmu*rstd))
                                    nmr = lsp.tile([P, 1], F32, tag="nmr")
                                    nc.vector.tensor_mul(nmr[:], negmu[:], rstd[:])
                                    nc.scalar.activation(xn[:], xt, AF.Identity,
                                                         scale=rstd[:], bias=nmr[:])
                                else:
                                    nc.vector.tensor_scalar(
                                        out=xn[:], in0=xt, scalar1=negmu[:],
                                        scalar2=rstd[:], op0=OP.add, op1=OP.mult)
                                pt = ptp.tile([P, DIM], MMDT, tag="pt")
                                for j in range(NT_D):
                                    nc.tensor.transpose(pt[:, j * P:(j + 1) * P],
                                                        xn[:, j * P:(j + 1) * P],
                                                        identr[:])
                                dst = xnT[:, :, i * P:(i + 1) * P]
                                psrc = pt[:].rearrange("p (j c) -> p j c", j=NT_D)
                                if i % 2 == 0:
                                    nc.scalar.activation(dst, psrc, AF.Copy)
                                else:
                                    nc.vector.tensor_copy(out=dst, in_=psrc)
                            # projections for this s-quarter
                            q0 = quarter * 512
                            for mc in range(2):
                                pq = ppp.tile([P, 512], F32, tag="s")
                                for cc in range(NT_D):
                                    nc.tensor.matmul(
                                        pq[:],
                                        lhsT=wq_sb[:, cc, mc * P:(mc + 1) * P],
                                        rhs=xnT[:, cc, q0:q0 + 512],
                                        start=(cc == 0), stop=(cc == NT_D - 1))
                                he, ho = 2 * mc, 2 * mc + 1
                                nc.vector.tensor_scalar_add(
                                    qTz[0:DH, he, q0:q0 + 512], pq[0:DH, :],
                                    qb_sb[0:DH, mc:mc + 1])
                                nc.scalar.activation(
                                    qTz[DH:P, ho, q0:q0 + 512], pq[DH:P, :],
                                    AF.Identity, bias=qb_sb[DH:P, mc:mc + 1])
                            pkv = ppp.tile([P, 512], F32, tag="s")
                            for cc in range(NT_D):
                                nc.tensor.matmul(
                                    pkv[:],
                                    lhsT=wkv_sb[:, cc, :],
                                    rhs=xnT[:, cc, q0:q0 + 512],
                                    start=(cc == 0), stop=(cc == NT_D - 1))
                            nc.vector.tensor_scalar_add(
                                kz[0:DH, q0:q0 + 512], pkv[0:DH, :],
                                kvb_sb[0:DH, 0:1])
                            nc.scalar.activation(
                                vsb[DH:P, q0:q0 + 512], pkv[DH:P, :],
                                AF.Identity, bias=kvb_sb[DH:P, 0:1])
                            # k^T upper-half copy + V natural for this quarter
                            nc.sync.dma_start(out=kdupz[DH:P, q0:q0 + 512],
                                              in_=kz[0:DH, q0:q0 + 512])
                            for m in range(quarter * 4, quarter * 4 + 4):
                                pv_t = ppp.tile([P, P], MMDT, tag="s")
                                nc.tensor.transpose(
                                    pv_t[:], vsb[:, m * P:(m + 1) * P], identr[:])
                                nc.vector.tensor_copy(out=vones[:, m, 0:DH],
                                                      in_=pv_t[:, DH:P])

            # ---------- Phase 2: main attention (k = 2..7) ----------
            with tc.tile_pool(name="att", bufs=3) as ap_, \
                 tc.tile_pool(name="bc", bufs=2) as bp, \
                 tc.tile_pool(name="psS", bufs=2, space="PSUM") as psp, \
                 tc.tile_pool(name="psO", bufs=2, space="PSUM") as pop:
                state.update(psp=psp, pop=pop, ap=ap_, bp=bp)
                for k in range(8):
                    for m in range(NT_S):
                        emit_step(k, m)
                flush_pv()

            # ---------- Phase 3: out-projection (chunked, pipelined) ----------
            with tc.tile_pool(name="yt", bufs=4) as yp, \
                 tc.tile_pool(name="psY", bufs=3, space="PSUM") as pyp:
                for mc in range(NT_D):
                    for half in range(2):
                        n0 = half * 1024
                        py = pyp.tile([P, 1024], F32, tag="py")
                        for ec in range(2):
                            for n in range(2):
                                nc.tensor.matmul(
                                    py[:, n * 512:(n + 1) * 512],
                                    lhsT=wo_sb[:, ec, mc * P:(mc + 1) * P],
                                    rhs=outT[:, ec, n0 + n * 512:n0 + (n + 1) * 512],
                                    start=(ec == 0), stop=(ec == 1))
                        yt = yp.tile([P, 1024], F32, tag="yt")
                        if half == 0:
                            nc.scalar.activation(yt[:], py[:], AF.Copy)
                        else:
                            nc.vector.tensor_copy(out=yt[:], in_=py[:])
                        nc.sync.dma_start(
                            out=y[mc * P:(mc + 1) * P, n0:n0 + 1024],
                            in_=yt[:])

    nc.compile()
    return nc


_NC = None


def _get_nc():
    global _NC
    if _NC is None:
        _NC = build_nc()
    return _NC


def make_in_maps(x, ln_gamma, ln_beta, w_q, w_k, w_v, w_o):
    x = np.asarray(x, np.float32)
    g_ = np.asarray(ln_gamma, np.float32)
    b_ = np.asarray(ln_beta, np.float32)
    w_o = np.asarray(w_o, np.float32)
    in_maps = []
    for core in range(8):
        b, g = divmod(core, 4)
        wq_s = np.ascontiguousarray(g_[:, None] * w_q[:, g * EG:(g + 1) * EG]
                                    ).astype(np.float16)
        wkv_s = np.concatenate(
            [g_[:, None] * w_k[:, g * DH:(g + 1) * DH],
             g_[:, None] * w_v[:, g * DH:(g + 1) * DH]], axis=1).astype(np.float16)
        qb_s = (b_ @ w_q[:, g * EG:(g + 1) * EG]).reshape(2, P).astype(np.float32)
        kvb_s = np.concatenate(
            [b_ @ w_k[:, g * DH:(g + 1) * DH],
             b_ @ w_v[:, g * DH:(g + 1) * DH]]).reshape(1, P).astype(np.float32)
        in_maps.append({
            "x": np.ascontiguousarray(x[b]),
            "wq": wq_s, "wkv": np.ascontiguousarray(wkv_s),
            "wo": np.ascontiguousarray(
                w_o[g * EG:(g + 1) * EG, :]).astype(np.float16),
            "qb": qb_s, "kvb": kvb_s,
        })
    return in_maps


def kernel(x, ln_gamma, ln_beta, w_q, w_k, w_v, w_o, b_o):
    nc = _get_nc()
    in_maps = make_in_maps(x, ln_gamma, ln_beta, w_q, w_k, w_v, w_o)
    res = run_bass_kernel_spmd(nc, in_maps, list(range(8)))
    out = np.zeros((B, S, DIM), np.float32)
    for core in range(8):
        b = core // 4
        out[b] += res.results[core]["y"].T
    out += np.asarray(b_o, np.float32)
    return out
